# revision 28
# baseline (speedup 1.0000x reference)
"""Trainium2 Bass kernel for nn_CMDTLoss (supervised-contrastive loss over
FFT'd features).

Math note: for real inputs, Parseval gives
    Re(fft(x) . conj(fft(y))) = D * (x . y)   and   ||fft(x)|| = sqrt(D)*||x||
so the cosine similarity of the FFT'd features equals the cosine similarity
of the raw features — the FFT cancels exactly. The loss is a SupCon loss on
plain cosine similarity.

Sharding: anchors (rows of the 4096x4096 sim matrix) are sharded across the
8 cores (512 rows each). Each core receives the full (normalized, transposed)
feature matrix with its columns ROTATED so that its own row-block is local
column block 0 — this makes the diagonal position identical on every core,
so one shared SPMD program works for all 8.

Per core:
  - sim row-block via fp8e4 DoubleRow matmuls (2 fp8/cell, PSUM fp32
    accumulation); fp8 quantization error averages out over the 4096-row
    mean (measured ~2.5e-5 relative on the final scalar)
  - denominator: fused exp(10*cos)+row-sum on ScalarE (one activation per
    1024/1536-wide tile via accum_out), diagonal removed by multiplying the
    diag 128x128 block by a device-generated (1-eye) before exp and
    subtracting exp(0)=1 from the row sum
  - numerator: sum of sim over same-label pairs via class-sum matmuls
    Z = onehot^T @ Y (classes x D, fp8 DoubleRow), Zg = onehot_block @ Z,
    then a fused multiply+row-reduce  s1_i = 10 * sum_d Y[i,d]*Zg[i,d];
    self term q_i = 10 * sum_d Y[i,d]^2 subtracted
  - per-row result r_i = (C_i * log(A_i) - (s1_i - q_i)) * w_i   (= -mlpp_i)
Host: shard/rotate + normalize inputs, loss = mean(all r_i).
"""

import sys

import numpy as np

_TRN_REPO = "/opt/trn_rl_repo"
if _TRN_REPO not in sys.path:
    sys.path.insert(0, _TRN_REPO)

N = 4096
D = 512
NCORES = 8
R = N // NCORES          # rows per core = 512
NCLS = 100
MCH = R // 128           # m-chunks per core = 4
NCH = N // 512           # n-chunks = 8
KCH = D // 128           # k-chunks = 4
NCLS_PAD = 112        # classes padded for DoubleRow weight stride %16
TEMP_INV = 10.0
EPS = 1e-8

_cache = {}


def _patch_act_tables():
    """Force the act-table-load pass to use natural_log_exp_and_others for
    both Exp and Ln (one table load, no mid-kernel Exp<->Ln set switch).
    Entry positions are preserved so act_func_set_id stays valid; every
    other set just advertises no functions."""
    import concourse.bacc as bacc_mod
    import concourse.hw_specs as hw
    if getattr(bacc_mod, "_cmdt_act_patch", False):
        return
    real = hw.get_activation_tables

    def patched(module_arch):
        tabs = real(module_arch)
        out = {}
        for name, fns in tabs.items():
            out[name] = fns if name == "natural_log_exp_and_others" else set()
        return out

    bacc_mod.get_activation_tables = patched
    bacc_mod._cmdt_act_patch = True


def _build_module():
    import concourse.tile as tile
    from concourse import bacc, mybir

    _patch_act_tables()

    bf16 = mybir.dt.bfloat16
    fp8 = mybir.dt.float8e4
    f32 = mybir.dt.float32
    Alu = mybir.AluOpType
    Act = mybir.ActivationFunctionType

    nc = bacc.Bacc("TRN2", target_bir_lowering=False, debug=False,
                   num_devices=NCORES)

    # DRAM I/O (per-core tensors, same names on every core)
    ytp = nc.dram_tensor("ytp", [128, NCH * KCH * 512], fp8,
                         kind="ExternalInput").ap()   # [p, (n,k,j)] col-rotated Y^T
    yp = nc.dram_tensor("yp", [128, (N // 128) * D], fp8,
                        kind="ExternalInput").ap()    # [p, (a,d)] row-rotated Y
    ohp = nc.dram_tensor("ohp", [128, (N // 128) * NCLS_PAD], fp8,
                         kind="ExternalInput").ap()   # [p, (a,c)] row-rotated onehot
    ohtb = nc.dram_tensor("ohtb", [NCLS, R], bf16,
                          kind="ExternalInput").ap()  # [c, local row]
    cvec = nc.dram_tensor("cvec", [128, MCH], f32,
                          kind="ExternalInput").ap()  # positives count per row
    wvec = nc.dram_tensor("wvec", [128, MCH], f32,
                          kind="ExternalInput").ap()  # 1/(C+eps) or 0
    res = nc.dram_tensor("res", [128, MCH], f32,
                         kind="ExternalOutput").ap()

    ACH = N // 128  # 32 row chunks for the Z matmuls

    with tile.TileContext(nc) as tc:
        with (
            tc.tile_pool(name="big", bufs=1) as big,
            tc.tile_pool(name="small", bufs=1) as small,
            tc.tile_pool(name="scratch", bufs=2) as scratch,
            tc.tile_pool(name="zps", bufs=1, space="PSUM") as zps,
            tc.tile_pool(name="simps", bufs=2, space="PSUM") as simps,
            tc.tile_pool(name="zgps", bufs=1, space="PSUM") as zgps,
        ):
            ytp_s = big.tile([128, NCH * KCH * 512], fp8, tag="ytp")
            yp_s = big.tile([128, ACH * D], fp8, tag="yp")
            ohp_s = big.tile([128, ACH * NCLS_PAD], fp8, tag="ohp")
            ohtb_s = small.tile([NCLS, R], bf16, tag="ohtb")
            zb_s = small.tile([NCLS, 512], bf16, tag="zb")
            cvec_s = small.tile([128, MCH], f32, tag="cvec")
            wvec_s = small.tile([128, MCH], f32, tag="wvec")
            asum_s = small.tile([128, MCH * 3], f32, tag="asum")
            s1_s = small.tile([128, MCH], f32, tag="s1")
            q_s = small.tile([128, MCH], f32, tag="q")
            res_s = small.tile([128, MCH], f32, tag="res")

            # --- input DMAs ---------------------------------------------
            # ytp (main GEMM operand) first; n-block 0 (which holds every
            # lhsT slice) in k-quarters so the first matmul group starts as
            # early as possible.  Z-path inputs after.
            NB = KCH * 512  # 2048 columns per n-block piece

            def dma_ytp_piece(n):
                nc.sync.dma_start(ytp_s[:, n * NB:(n + 1) * NB],
                                  ytp[:, n * NB:(n + 1) * NB])

            dumm = scratch.tile([128, 1], f32, tag="dumm")
            nc.vector.memset(dumm[:], 0.0)
            dscr = scratch.tile([128, 1], f32, tag="dumm")
            nc.scalar.activation(dscr[:], dumm[:], Act.Exp, bias=0.0,
                                 scale=1.0)

            # (1 - eye) built on device: idx[p,j] = j - p, then != 0
            idx_s = small.tile([128, 128], mybir.dt.int32, tag="idx")
            nc.gpsimd.iota(idx_s[:], pattern=[[1, 128]], base=0,
                           channel_multiplier=-1)
            eyem_s = small.tile([128, 128], f32, tag="eyem")
            nc.vector.tensor_scalar(out=eyem_s[:], in0=idx_s[:], scalar1=0,
                                    scalar2=None, op0=Alu.not_equal)

            cm1_s = small.tile([128, 1], f32, tag="cm1")
            nc.vector.memset(cm1_s[:], -1.0)
            warm_s = scratch.tile([128, 128], bf16, tag="warm")
            nc.vector.memset(warm_s[:], 0.0)
            wps = zgps.tile([128, 128], f32, tag="zg")
            for _ in range(24):
                nc.tensor.matmul(wps[:], lhsT=warm_s[:], rhs=warm_s[:],
                                 start=True, stop=True)

            # ytp pieces sized to balance DMA issue rate (~0.65us per DMA)
            # against arrival deadlines; Z-path inputs as single large DMAs.
            dma_ytp_piece(0)
            dma_ytp_piece(1)
            for n in (2, 4, 6):
                nc.sync.dma_start(ytp_s[:, n * NB:(n + 2) * NB],
                                  ytp[:, n * NB:(n + 2) * NB])
            nc.sync.dma_start(yp_s[:], yp[:])
            nc.sync.dma_start(ohp_s[:], ohp[:])
            nc.sync.dma_start(ohtb_s[:], ohtb[:])
            nc.sync.dma_start(cvec_s[:], cvec[:])
            nc.sync.dma_start(wvec_s[:], wvec[:])

            # --- main GEMM + fused exp/rowsum (n outer: block n is only
            # needed once its DMA piece has landed) -----------------------
            TILE_NB = [(0, 2), (2, 5), (5, 8)]  # col-block ranges per sim tile

            def main_group(t, m):
                nb0, nb1 = TILE_NB[t]
                width = (nb1 - nb0) * 512
                ps = simps.tile([128, width], f32, tag="sim")
                for h in range(nb1 - nb0):
                    nb = nb0 + h
                    for j in range(2):  # two DoubleRow matmuls: k-chunk pairs
                        lpair = ytp_s[:, 2 * j * 512:(2 * j + 2) * 512].rearrange(
                            "p (two n) -> p two n", two=2)
                        rpair = ytp_s[:, nb * NB + 2 * j * 512:
                                      nb * NB + (2 * j + 2) * 512].rearrange(
                            "p (two n) -> p two n", two=2)
                        nc.tensor.matmul(
                            ps[:, h * 512:(h + 1) * 512],
                            lhsT=lpair[:, :, m * 128:(m + 1) * 128],
                            rhs=rpair[:],
                            start=(j == 0), stop=(j == 1),
                            perf_mode=mybir.MatmulPerfMode.DoubleRow,
                        )
                    if t == 0 and h == 0:
                        # diag block lives in half 0 (bank 0): zero it while
                        # the remaining halves are still matmulling
                        blk = ps[:, m * 128:(m + 1) * 128]
                        nc.vector.tensor_tensor(blk, blk, eyem_s[:],
                                                op=Alu.mult)
                nc.scalar.activation(
                    ps[:], ps[:], Act.Exp, bias=0.0, scale=TEMP_INV,
                    accum_out=asum_s[:, m * 3 + t:m * 3 + t + 1],
                )

            for t in range(2):
                for m in range(MCH):
                    main_group(t, m)

            # q_m = 10 * sum_d y^2 (self-similarity term) — DVE, early
            for m in range(MCH):
                yblk = yp_s[:, m * D:(m + 1) * D]
                qscr = scratch.tile([128, D], f32, tag="qscr")
                nc.vector.scalar_tensor_tensor(
                    out=qscr[:], in0=yblk, scalar=TEMP_INV, in1=yblk,
                    op0=Alu.mult, op1=Alu.mult,
                    accum_out=q_s[:, m:m + 1],
                )

            main_group(2, 0)

            # --- Z = onehot^T @ Y  (classes x 512); interleaved between the
            # last sim tiles so the Zg->s1 DVE chain overlaps ACT's exps ---
            zpsum = zps.tile([NCLS_PAD, 512], f32, tag="z")
            for g in range(ACH // 2):
                opair = ohp_s[:, 2 * g * NCLS_PAD:(2 * g + 2) * NCLS_PAD].rearrange(
                    "p (two c) -> p two c", two=2)
                ypair = yp_s[:, 2 * g * D:(2 * g + 2) * D].rearrange(
                    "p (two d) -> p two d", two=2)
                nc.tensor.matmul(
                    zpsum[:],
                    lhsT=opair[:],
                    rhs=ypair[:],
                    start=(g == 0), stop=(g == ACH // 2 - 1),
                    perf_mode=mybir.MatmulPerfMode.DoubleRow,
                )
            nc.vector.tensor_copy(zb_s[:], zpsum[0:NCLS, :])  # fp32 -> bf16 cast

            def zg_chain(m):
                zg = zgps.tile([128, 512], f32, tag="zg")
                nc.tensor.matmul(
                    zg[:],
                    lhsT=ohtb_s[:, m * 128:(m + 1) * 128],
                    rhs=zb_s[:],
                    start=True, stop=True,
                )
                # s1_m = 10 * sum_d y * Zg
                sscr = scratch.tile([128, D], f32, tag="qscr")
                nc.vector.scalar_tensor_tensor(
                    out=sscr[:], in0=zg[:], scalar=TEMP_INV,
                    in1=yp_s[:, m * D:(m + 1) * D],
                    op0=Alu.mult, op1=Alu.mult,
                    accum_out=s1_s[:, m:m + 1],
                )

            zg_chain(0)
            main_group(2, 1)
            zg_chain(1)
            main_group(2, 2)
            zg_chain(2)
            main_group(2, 3)
            zg_chain(3)


            # --- finishing ----------------------------------------------
            # One reduce over the whole asum tile [128, (m,n)] -> [128, MCH]
            # (reads every exp's accum column, so it schedules after the
            # last Exp -> exactly one Exp->Ln ACT table switch).
            av = asum_s[:].rearrange("p (m n) -> p m n", n=3)
            a2 = small.tile([128, MCH], f32, tag="a2")
            nc.vector.tensor_reduce(a2[:], av[:, :, 0:2],
                                    axis=mybir.AxisListType.X, op=Alu.add)
            a_all = small.tile([128, MCH], f32, tag="a_all")
            nc.vector.tensor_tensor(
                a_all[:], a2[:],
                av[:, :, 2:3].rearrange("p m n -> p (m n)"), op=Alu.add)
            # A includes exp(0)=1 from the zeroed diagonal: ln(A - 1)
            loga = small.tile([128, MCH], f32, tag="loga")
            nc.scalar.activation(loga[:], a_all[:], Act.Ln,
                                 bias=cm1_s[:], scale=1.0)
            t_all = small.tile([128, MCH], f32, tag="t_all")
            nc.vector.tensor_sub(t_all[:], s1_s[:], q_s[:])
            u_all = small.tile([128, MCH], f32, tag="u_all")
            nc.vector.tensor_mul(u_all[:], cvec_s[:], loga[:])
            # res = (C*logA - (s1 - q)) * w   (= -masked-logprob-mean)
            nc.vector.tensor_sub(u_all[:], u_all[:], t_all[:])
            nc.vector.tensor_mul(res_s[:], u_all[:], wvec_s[:])

            nc.sync.dma_start(res[:], res_s[:])

    nc.compile()
    return nc


def _host_prep(features, labels):
    """Build per-core input maps."""
    import ml_dtypes
    bf16 = ml_dtypes.bfloat16

    fp8 = ml_dtypes.float8_e4m3
    feats = np.asarray(features, dtype=np.float32)
    labels = np.asarray(labels).astype(np.int64)

    norms = np.sqrt((feats.astype(np.float32) ** 2).sum(axis=1, keepdims=True))
    Y = (feats / norms).astype(bf16)                       # [N, D]
    Y8 = Y.astype(fp8)
    OH = (labels[:, None] == np.arange(NCLS)[None, :]).astype(bf16)  # [N, C]
    OH8 = np.zeros((N, NCLS_PAD), dtype=fp8)
    OH8[:, :NCLS] = OH.astype(fp8)

    counts = np.bincount(labels, minlength=NCLS)
    C = (counts[labels] - 1).astype(np.float32)            # positives per row
    W = np.where(C > 0, 1.0 / (C + EPS), 0.0).astype(np.float32)

    in_maps = []
    for c in range(NCORES):
        rot = np.roll(np.arange(N), -c * R)
        Yr = Y[rot]                                        # [N, D] row-rotated
        # ytp[p, n, k, j] = Yr[n*512+j, k*128+p]
        T = np.ascontiguousarray(Yr.T).astype(fp8)         # [D, N]
        ytp = np.ascontiguousarray(
            T.reshape(KCH, 128, NCH, 512).transpose(1, 2, 0, 3)
        ).reshape(128, NCH * KCH * 512)
        yp = np.ascontiguousarray(
            Y8[rot].reshape(N // 128, 128, D).transpose(1, 0, 2)
        ).reshape(128, (N // 128) * D)
        ohp = np.ascontiguousarray(
            OH8[rot].reshape(N // 128, 128, NCLS_PAD).transpose(1, 0, 2)
        ).reshape(128, (N // 128) * NCLS_PAD)
        ohtb = np.ascontiguousarray(OH[c * R:(c + 1) * R].T)  # [C, R]
        cvec = np.ascontiguousarray(
            C[c * R:(c + 1) * R].reshape(MCH, 128).T)         # [128, MCH]
        wvec = np.ascontiguousarray(
            W[c * R:(c + 1) * R].reshape(MCH, 128).T)
        in_maps.append({
            "ytp": ytp, "yp": yp, "ohp": ohp, "ohtb": ohtb,
            "cvec": cvec, "wvec": wvec,
        })
    return in_maps


def _get_nc():
    if "nc" not in _cache:
        _cache["nc"] = _build_module()
    return _cache["nc"]


def kernel(features, labels):
    from concourse.bass_utils import run_bass_kernel_spmd

    nc = _get_nc()
    in_maps = _host_prep(features, labels)
    out = run_bass_kernel_spmd(nc, in_maps, core_ids=list(range(NCORES)))
    vals = np.concatenate(
        [out.results[c]["res"].reshape(-1) for c in range(NCORES)])
    loss = np.asarray(vals, dtype=np.float64).mean()
    return np.float32(loss)
